# revision 26
# baseline (speedup 1.0000x reference)
"""Trainium2 Bass kernel for nn_CellDetectorWithClassifierInd.

Pipeline (B=4 images, 8 NeuronCores, data-parallel over half-images / roi halves):
  host   : patch-head MLP + softmax + bilinear resize in float64 (tiny: 1.6 GFLOP)
           -> bad mask + uncertainty band (needed to make the discrete mask
              decisions verifiable against the fp32 reference).
  stage A: (device) belief-map NMS: 7x7 separable max-pool, peak candidates,
           4x4 block max + argmax-index reduction.  Core c handles half of
           image c//2 (256 rows + 3-row halo).
  host   : merge candidates, exact top-k with lax.top_k tie-breaking, safety
           guards (falls back to a full jax reference recompute if any
           ambiguity could flip the selected peak set).
  stage B: (device) roi-align gather (dynamic-offset DMA from C-major feature
           map, 11x11 patches), bilinear pooling as matmul, 3x 3x3-conv head
           (float32r matmuls on the PE), spatial max, final FC.
  host   : assemble outputs.
"""

import numpy as np
import concourse.bass as bass
import concourse.mybir as mybir
from concourse.tile import TileContext
from concourse.bass_utils import run_bass_kernel_spmd
from concourse.masks import make_identity

f32 = mybir.dt.float32
f32r = mybir.dt.float32r
i32 = mybir.dt.int32

B, H, W = 4, 512, 512
DH = DW = 32
K = 512
NMS_THR = 0.2
MIN_DIST = 3
PROP_HALF = 5
POOL = 7
NCLS = 4
C_UP = 64

NEG = -1.0e38
GATE = 2.0e38

N_CORES = 8
SLAB = 262          # 256 rows + 3 halo each side
NROI = 264          # 256 real rois + 8 dummy (divisible by 8 and 12)
FEAT_ROWS = 528     # 512 image rows + 16 appendix rows for border-roi patches
APP_STRIDE = 16     # appendix packing stride (collision-free: |16*dk + dc| < 512)
MAX_BORDER = 31

SCORE_BAND = 2.0e-6  # |score64 - 0.5| band treated as uncertain (measured
                     # |score64 - score_ref_fp32| max ~2.7e-7, 7x margin)


# --------------------------------------------------------------------------
# BIR post-pass: split instructions with >1 semaphore waits (TRN2 walrus
# CoreV3 rejects multi-wait Drains emitted by the Tile kernel-tail barrier).
# --------------------------------------------------------------------------
def _fix_multiwait(nc, maxw=1):
    for bb in nc.m.functions[0].blocks:
        newlist = []
        for ins in bb.instructions:
            si = ins.sync_info
            if si is not None and si.on_wait is not None and len(si.on_wait) > maxw:
                waits = list(si.on_wait)
                for w in waits[maxw:]:
                    nop = mybir.InstNoOp(name=nc.get_next_instruction_name(), ins=[], outs=[])
                    nop.engine = ins.engine
                    nop.sync_info = mybir.SyncInfo(on_wait=[w], on_update=[])
                    newlist.append(nop)
                si.on_wait = waits[:maxw]
            newlist.append(ins)
        bb.instructions = newlist


# --------------------------------------------------------------------------
# Host: float64 patch-score path
# --------------------------------------------------------------------------
def _resize_matrix_1d(n_out, n_in):
    """Row-normalized triangle-kernel weight matrix matching
    jax.image.resize(..., method='bilinear') with align_corners=False."""
    scale = n_in / n_out
    i = np.arange(n_out, dtype=np.float64)
    x = (i + 0.5) * scale - 0.5          # sample position in input coords
    j = np.arange(n_in, dtype=np.float64)
    w = np.maximum(0.0, 1.0 - np.abs(x[:, None] - j[None, :]))
    s = w.sum(axis=1, keepdims=True)
    return w / s


def _host_score64(feat_down, pw1, pb1, pw2, pb2, pw3, pb3, pfc_w, pfc_b):
    X = feat_down.reshape(B, 256, DH * DW).astype(np.float64)

    def lk(v):
        return np.where(v > 0, v, 0.1 * v)

    f = lk(np.matmul(pw1.astype(np.float64), X) + pb1.astype(np.float64)[None, :, None])
    f = lk(np.matmul(pw2.astype(np.float64), f) + pb2.astype(np.float64)[None, :, None])
    f = lk(np.matmul(pw3.astype(np.float64), f) + pb3.astype(np.float64)[None, :, None])
    pl = np.matmul(pfc_w.astype(np.float64), f) + pfc_b.astype(np.float64)[None, :, None]
    d = pl[:, 1] - pl[:, 0]
    score32 = (1.0 / (1.0 + np.exp(-d))).reshape(B, DH, DW)
    Wm = _resize_matrix_1d(H, DH)        # [512, 32]
    s512 = np.einsum("yi,bij,xj->byx", Wm, score32, Wm, optimize=True)
    return s512                           # [B, 512, 512] float64


# --------------------------------------------------------------------------
# Host: full reference recompute in jax (CPU) -- correctness fallback
# --------------------------------------------------------------------------
def _full_reference_fallback(inputs):
    import jax
    import jax.numpy as jnp
    from jax import lax

    cpu = jax.devices("cpu")[0]
    with jax.default_device(cpu):
        xhat = jnp.asarray(inputs["xhat"])
        feat_down = jnp.asarray(inputs["feat_down"])
        feat_up = jnp.asarray(inputs["feat_up"])

        def leaky(x):
            return jnp.where(x > 0, x, 0.1 * x)

        f = leaky(jnp.einsum('bchw,oc->bohw', feat_down, inputs["pw1"]) + inputs["pb1"][None, :, None, None])
        f = leaky(jnp.einsum('bchw,oc->bohw', f, inputs["pw2"]) + inputs["pb2"][None, :, None, None])
        f = leaky(jnp.einsum('bchw,oc->bohw', f, inputs["pw3"]) + inputs["pb3"][None, :, None, None])
        plogits = jnp.einsum('bchw,oc->bohw', f, inputs["pfc_w"]) + inputs["pfc_b"][None, :, None, None]
        patch_score = jax.nn.softmax(plogits, axis=1)[:, 1:2]
        patch_score = jax.image.resize(patch_score, (B, 1, H, W), 'bilinear')
        bad = patch_score < 0.5
        nbad = jnp.maximum(jnp.sum(bad), 1).astype(xhat.dtype)
        mean_bad = jnp.sum(jnp.where(bad, xhat, 0.0)) / nbad
        bm = jnp.where(bad, mean_bad, xhat)[:, 0]
        win = 2 * MIN_DIST + 1
        mp = lax.reduce_window(bm, -jnp.inf, lax.max, (1, win, win), (1, 1, 1), 'SAME')
        peak_scores = jnp.where((bm == mp) & (bm > NMS_THR), bm, -jnp.inf)
        top_s, top_i = lax.top_k(peak_scores.reshape(B, -1), K)
        ys = (top_i // W).astype(jnp.int32)
        xs = (top_i % W).astype(jnp.int32)
        valid = top_s > NMS_THR
        coords = jnp.stack([xs, ys], axis=-1)
        roi_size = 2.0 * PROP_HALF
        offs = (jnp.arange(POOL, dtype=jnp.float32) + 0.5) * (roi_size / POOL)
        px = (xs.astype(jnp.float32) - PROP_HALF)[..., None] + offs
        py = (ys.astype(jnp.float32) - PROP_HALF)[..., None] + offs

        def sample_one(feat, py_i, px_i):
            yy = jnp.broadcast_to(py_i[:, :, None], (K, POOL, POOL))
            xx = jnp.broadcast_to(px_i[:, None, :], (K, POOL, POOL))
            y0 = jnp.floor(yy); x0 = jnp.floor(xx)
            wy = yy - y0; wx = xx - x0
            y0i = jnp.clip(y0.astype(jnp.int32), 0, H - 1)
            y1i = jnp.clip(y0.astype(jnp.int32) + 1, 0, H - 1)
            x0i = jnp.clip(x0.astype(jnp.int32), 0, W - 1)
            x1i = jnp.clip(x0.astype(jnp.int32) + 1, 0, W - 1)
            g = lambda yi, xi: feat[:, yi, xi]
            return (g(y0i, x0i) * (1 - wy) * (1 - wx) + g(y0i, x1i) * (1 - wy) * wx
                    + g(y1i, x0i) * wy * (1 - wx) + g(y1i, x1i) * wy * wx)

        pooled = jax.vmap(sample_one)(feat_up, py, px)
        pooled = pooled.transpose(0, 2, 1, 3, 4).reshape(B * K, C_UP, POOL, POOL)
        dn = ('NCHW', 'OIHW', 'NCHW')
        from jax import lax as _lax
        h = leaky(_lax.conv_general_dilated(pooled, inputs["qw1"], (1, 1), 'SAME', dimension_numbers=dn) + inputs["qb1"][None, :, None, None])
        h = leaky(_lax.conv_general_dilated(h, inputs["qw2"], (1, 1), 'SAME', dimension_numbers=dn) + inputs["qb2"][None, :, None, None])
        h = leaky(_lax.conv_general_dilated(h, inputs["qw3"], (1, 1), 'SAME', dimension_numbers=dn) + inputs["qb3"][None, :, None, None])
        h = jnp.max(h, axis=(2, 3))
        cls_logits = (h @ inputs["qfc_w"].T + inputs["qfc_b"]).reshape(B, K, NCLS)
        return (np.asarray(cls_logits), np.asarray(coords), np.asarray(top_s), np.asarray(valid))


# --------------------------------------------------------------------------
# Stage A device program: NMS + candidate block-reduction
# --------------------------------------------------------------------------
def _build_stage_a():
    nc = bass.Bass()
    m_in = nc.dram_tensor("m_in", [SLAB, W], f32, kind="ExternalInput")
    idx_in = nc.dram_tensor("idx_in", [256, W], f32, kind="ExternalInput")
    bvals = nc.dram_tensor("bvals", [64, 128], f32, kind="ExternalOutput")
    bidx = nc.dram_tensor("bidx", [64, 128], f32, kind="ExternalOutput")
    mx_d = nc.dram_tensor("mx_d", [SLAB, W], f32)
    cand_d = nc.dram_tensor("cand_d", [256, W], f32)

    with TileContext(nc) as tc:
        with tc.tile_pool(name="sb", bufs=2) as pool, \
             tc.tile_pool(name="sb1", bufs=1) as pool1:
            # ---- phase 1: x-direction 7-window max
            for c0, rows in ((0, 128), (128, 128), (256, 6)):
                pad = pool.tile([128, W + 6], f32, tag="pad")
                nc.vector.memset(pad[:rows, :], NEG)
                nc.sync.dma_start(pad[:rows, 3:W + 3], m_in[c0:c0 + rows, :])
                mx = pool.tile([128, W], f32, tag="mx")
                nc.vector.tensor_max(mx[:rows, :], pad[:rows, 0:W], pad[:rows, 1:W + 1])
                for d in range(2, 7):
                    nc.vector.tensor_max(mx[:rows, :], mx[:rows, :], pad[:rows, d:W + d])
                nc.sync.dma_start(mx_d[c0:c0 + rows, :], mx[:rows, :])
            # ---- phase 2: y-direction 7-window max + candidate mask
            for j0, rows in ((0, 120), (120, 120), (240, 16)):
                acc = pool.tile([128, W], f32, tag="yacc")
                nc.sync.dma_start(acc[:rows, :], mx_d[j0:j0 + rows, :])
                for k in range(1, 7):
                    tk = pool.tile([128, W], f32, tag="yld")
                    nc.sync.dma_start(tk[:rows, :], mx_d[j0 + k:j0 + k + rows, :])
                    nc.vector.tensor_max(acc[:rows, :], acc[:rows, :], tk[:rows, :])
                m_t = pool.tile([128, W], f32, tag="mrow")
                nc.sync.dma_start(m_t[:rows, :], m_in[j0 + 3:j0 + 3 + rows, :])
                e01 = pool.tile([128, W], f32, tag="e01")
                nc.vector.tensor_tensor(out=e01[:rows, :], in0=m_t[:rows, :], in1=acc[:rows, :],
                                        op=mybir.AluOpType.is_equal)
                g01 = pool.tile([128, W], f32, tag="g01")
                nc.vector.tensor_scalar(out=g01[:rows, :], in0=m_t[:rows, :], scalar1=float(NMS_THR),
                                        scalar2=None, op0=mybir.AluOpType.is_gt)
                c5 = pool.tile([128, W], f32, tag="c5")
                nc.vector.tensor_tensor(out=c5[:rows, :], in0=e01[:rows, :], in1=g01[:rows, :],
                                        op=mybir.AluOpType.mult)
                nc.vector.tensor_scalar(out=c5[:rows, :], in0=c5[:rows, :], scalar1=0.5, scalar2=None,
                                        op0=mybir.AluOpType.subtract)
                cand = pool.tile([128, W], f32, tag="cand")
                nc.vector.scalar_tensor_tensor(out=cand[:rows, :], in0=c5[:rows, :], scalar=GATE,
                                               in1=m_t[:rows, :], op0=mybir.AluOpType.mult,
                                               op1=mybir.AluOpType.min)
                nc.sync.dma_start(cand_d[j0:j0 + rows, :], cand[:rows, :])
            # ---- phase 3: 4x4 block max + argmax index
            bl = pool1.tile([64, 2048], f32, tag="bl")
            il = pool1.tile([64, 2048], f32, tag="il")
            for rr in range(4):
                src = cand_d.ap().rearrange("(q four) n -> q four n", four=4)[:, rr, :]
                nc.sync.dma_start(bl[:, rr * 512:(rr + 1) * 512].rearrange("p (a c) -> p a c", c=4),
                                  src.rearrange("q (a c) -> q a c", c=4))
                srci = idx_in.ap().rearrange("(q four) n -> q four n", four=4)[:, rr, :]
                nc.sync.dma_start(il[:, rr * 512:(rr + 1) * 512].rearrange("p (a c) -> p a c", c=4),
                                  srci.rearrange("q (a c) -> q a c", c=4))
            bl4 = bl[:].rearrange("p (b a c) -> p a b c", b=4, c=4)
            bmax = pool1.tile([64, 128], f32, tag="bmax")
            nc.vector.tensor_reduce(out=bmax[:], in_=bl4, axis=mybir.AxisListType.XY,
                                    op=mybir.AluOpType.max)
            nc.sync.dma_start(bvals[:], bmax[:])
            eqt = pool1.tile([64, 2048], f32, tag="eqt")
            eq4 = eqt[:].rearrange("p (b a c) -> p a b c", b=4, c=4)
            bb = bmax[:].to_broadcast([64, 128, 4, 4])
            nc.vector.tensor_tensor(out=eq4, in0=bl4, in1=bb, op=mybir.AluOpType.is_equal)
            nc.vector.tensor_scalar(out=eqt[:], in0=eqt[:], scalar1=0.5, scalar2=None,
                                    op0=mybir.AluOpType.subtract)
            isel = pool1.tile([64, 2048], f32, tag="isel")
            nc.vector.scalar_tensor_tensor(out=isel[:], in0=eqt[:], scalar=-GATE,
                                           in1=il[:], op0=mybir.AluOpType.mult,
                                           op1=mybir.AluOpType.max)
            bai = pool1.tile([64, 128], f32, tag="bai")
            nc.vector.tensor_reduce(out=bai[:], in_=isel[:].rearrange("p (b a c) -> p a b c", b=4, c=4),
                                    axis=mybir.AxisListType.XY, op=mybir.AluOpType.min)
            nc.sync.dma_start(bidx[:], bai[:])

    nc.finalize()
    _fix_multiwait(nc)
    return nc


# --------------------------------------------------------------------------
# Stage B device program: roi gather + pooling + conv head
# --------------------------------------------------------------------------
def _pool_matrix():
    """Shared [121, 49] bilinear pooling matrix (canonical fractional weights)."""
    offs = (np.arange(POOL, dtype=np.float32) + np.float32(0.5)) * np.float32(2.0 * PROP_HALF / POOL)
    u0 = np.floor(offs).astype(np.int64)
    fr = (offs - u0.astype(np.float32)).astype(np.float32)
    M = np.zeros((121, 49), dtype=np.float32)
    for i in range(POOL):
        for j in range(POOL):
            s = i * 7 + j
            for du, wu in ((0, np.float32(1.0) - fr[i]), (1, fr[i])):
                for dv, wv in ((0, np.float32(1.0) - fr[j]), (1, fr[j])):
                    t = (u0[i] + du) * 11 + (u0[j] + dv)
                    M[t, s] += wu * wv
    return M


def _build_stage_b(dt_conv=f32r):
    nc = bass.Bass()
    patches = nc.dram_tensor("patches", [64, NROI * 121], f32, kind="ExternalInput")
    mpool_in = nc.dram_tensor("mpool", [121, 49], f32, kind="ExternalInput")
    qw1_in = nc.dram_tensor("qw1p", [64, 9 * 256], f32, kind="ExternalInput")
    qw2_in = nc.dram_tensor("qw2p", [128, 18 * 256], f32, kind="ExternalInput")
    qw3_in = nc.dram_tensor("qw3p", [128, 18 * 256], f32, kind="ExternalInput")
    qfc_in = nc.dram_tensor("qfcp", [128, 8], f32, kind="ExternalInput")
    qb_in = nc.dram_tensor("qbp", [1, 3 * 256], f32, kind="ExternalInput")   # qb1|qb2|qb3
    qfcb_in = nc.dram_tensor("qfcb", [4, 1], f32, kind="ExternalInput")
    lg_out = nc.dram_tensor("lg", [4, NROI], f32, kind="ExternalOutput")

    NB = NROI // 8     # pooling batches
    NG = NROI // 12    # conv groups

    with TileContext(nc) as tc:
        with tc.tile_pool(name="cst", bufs=1) as cpool, \
             tc.tile_pool(name="wrk", bufs=3) as pool, \
             tc.tile_pool(name="act", bufs=2) as apool, \
             tc.tile_pool(name="ps_t", bufs=1, space="PSUM") as ps_t, \
             tc.tile_pool(name="ps_p", bufs=1, space="PSUM") as ps_p, \
             tc.tile_pool(name="ps_r", bufs=1, space="PSUM") as ps_r, \
             tc.tile_pool(name="ps_c", bufs=2, space="PSUM") as ps_c:
            ident = cpool.tile([128, 128], f32)
            make_identity(nc, ident)
            mp_t = cpool.tile([121, 49], f32, tag="mp")
            nc.sync.dma_start(mp_t[:], mpool_in[:])
            qw1_t = cpool.tile([64, 9 * 256], dt_conv, tag="qw1")
            nc.gpsimd.dma_start(qw1_t[:], qw1_in[:])
            qw2_t = cpool.tile([128, 18 * 256], dt_conv, tag="qw2")
            nc.gpsimd.dma_start(qw2_t[:], qw2_in[:])
            qw3_t = cpool.tile([128, 18 * 256], dt_conv, tag="qw3")
            nc.gpsimd.dma_start(qw3_t[:], qw3_in[:])
            qfc_t = cpool.tile([128, 8], dt_conv, tag="qfc")
            nc.gpsimd.dma_start(qfc_t[:], qfc_in[:])
            qb_t = cpool.tile([1, 3 * 256], dt_conv, tag="qb")
            nc.gpsimd.dma_start(qb_t[:], qb_in[:])
            qfcb_t = cpool.tile([4, 1], f32, tag="qfcb")
            nc.sync.dma_start(qfcb_t[:], qfcb_in[:])
            ones_t = cpool.tile([1, 486], dt_conv, tag="ones")
            nc.vector.memset(ones_t[:].bitcast(f32), 1.0)


            # GUARD elements on each side so tap-shifted contiguous reads stay in-tile
            GUARD = 16
            pooled = cpool.tile([64, GUARD + NROI * 81 + GUARD], dt_conv, tag="pooled")
            nc.vector.memset(pooled[:].bitcast(f32), 0.0)
            h0 = cpool.tile([128, NROI], dt_conv, tag="h0")
            h1 = cpool.tile([128, NROI], dt_conv, tag="h1")
            h_t = (h0, h1)

            def emit_pool_batch(bi):
                batch = pool.tile([121, 512], f32, tag="batch")
                pt8 = pool.tile([64, 8 * 121], f32, tag="pt8")
                nc.sync.dma_start(pt8[:], patches[:, bi * 8 * 121:(bi + 1) * 8 * 121])
                for s in range(8):
                    pt_ps = ps_t.tile([121, 64], f32, tag="tp")
                    nc.tensor.transpose(pt_ps[:], pt8[:, s * 121:(s + 1) * 121], ident[:64, :64])
                    nc.vector.tensor_copy(batch[:, s * 64:(s + 1) * 64], pt_ps[:])
                pl_ps = ps_p.tile([49, 512], f32, tag="pp")
                nc.tensor.matmul(pl_ps[:], lhsT=mp_t[:], rhs=batch[:], start=True, stop=True)
                pl_sb = pool.tile([49, 512], f32, tag="plsb")
                nc.vector.tensor_copy(pl_sb[:], pl_ps[:])
                for s in range(8):
                    r = bi * 8 + s
                    rt_ps = ps_r.tile([64, 49], f32, tag="rp")
                    nc.tensor.transpose(rt_ps[:], pl_sb[:, s * 64:(s + 1) * 64], ident[:49, :49])
                    base = GUARD + r * 81 + 10
                    dst = pooled[:, base:base + 63].rearrange("c (r w) -> c r w", r=7, w=9)[:, :, :7]
                    nc.vector.tensor_copy(dst, rt_ps[:].rearrange("c (r w) -> c r w", r=7, w=7))

            # Conv layers compute the FULL padded 9x9 grid per roi (N = 6*81 = 486
            # per matmul): every tap reads a CONTIGUOUS 486-window at a +-10
            # shifted offset (fp32r matmuls reject non-contiguous moving APs).
            # Ring outputs accumulate garbage and are simply never read.
            VIEW49 = ("c (r a b) -> c r a b", dict(r=6, a=9))

            def valid49(tile_ap, elem_base):
                """[P, 6, 7, 7] view of the 49 valid positions of 6 rois' 81-maps."""
                v = tile_ap[:, elem_base:elem_base + 6 * 81].rearrange("c (r a b) -> c r a b", r=6, a=9)
                return v[:, :, 1:8, 1:8]

            def conv_layer(src_of, wt, wcols_fn, bias_col0, dst_fn, kchunks):
                # one conv layer for a 12-roi group processed as 2 halves of 6
                for hh in range(2):
                    for m in range(2):
                        psum = ps_c.tile([128, 486], f32, tag=f"cm{m}")
                        nc.tensor.matmul(psum[:], lhsT=qb_t[0:1, bias_col0 + 128 * m:bias_col0 + 128 * m + 128],
                                         rhs=ones_t[:], start=True, stop=False)
                        n_mm = kchunks * 9
                        cnt = 0
                        for kk in range(kchunks):
                            tile_ap, base = src_of(kk, hh)    # base = element idx of roi0's 81-map
                            for t in range(9):
                                dy, dx = t // 3, t % 3
                                off = base + 9 * (dy - 1) + (dx - 1)
                                rhs = tile_ap[:, off:off + 486]
                                cnt += 1
                                nc.tensor.matmul(psum[:], lhsT=wt[:, wcols_fn(kk, t, m):wcols_fn(kk, t, m) + 128],
                                                 rhs=rhs, start=False, stop=(cnt == n_mm))
                        dst_fn(hh, m, psum)

            def leaky_to(dst49, psum):
                """dst49: [128, 6, 7, 7] SBUF view. psum: [128, 486] full grid."""
                pv = psum[:].rearrange("c (r a b) -> c r a b", r=6, a=9)[:, :, 1:8, 1:8]
                tmp = pool.tile([128, 294], f32, tag="lk", name=f"lk{nc.next_id()}")
                t4 = tmp[:].rearrange("c (r a b) -> c r a b", r=6, a=7)
                nc.scalar.mul(t4, pv, 0.1)
                nc.vector.tensor_tensor(out=dst49, in0=t4, in1=pv, op=mybir.AluOpType.max)

            ACT_GSZ = GUARD + 12 * 81 + GUARD

            def emit_conv_group(g):
                # conv1: input = pooled rois [12g .. 12g+11]
                act1 = [apool.tile([128, ACT_GSZ], dt_conv, name=f"act1_{g}_{m}", tag=f"a1k{m}") for m in range(2)]
                act2 = [apool.tile([128, ACT_GSZ], dt_conv, name=f"act2_{g}_{m}", tag=f"a2k{m}") for m in range(2)]
                for m in range(2):
                    nc.vector.memset(act1[m][:].bitcast(f32), 0.0)
                    nc.vector.memset(act2[m][:].bitcast(f32), 0.0)

                def dst1(hh, m, psum):
                    leaky_to(valid49(act1[m], GUARD + hh * 6 * 81), psum)

                conv_layer(lambda kk, hh: (pooled, GUARD + (g * 12 + hh * 6) * 81),
                           qw1_t, lambda kk, t, m: t * 256 + 128 * m, 0, dst1, 1)

                def dst2(hh, m, psum):
                    leaky_to(valid49(act2[m], GUARD + hh * 6 * 81), psum)

                conv_layer(lambda kk, hh: (act1[kk], GUARD + hh * 6 * 81),
                           qw2_t, lambda kk, t, m: (kk * 9 + t) * 256 + 128 * m, 256, dst2, 2)

                # conv3 + leaky + spatial max -> h
                def dst3(hh, m, psum):
                    a3 = pool.tile([128, 294], dt_conv, tag="a3")
                    leaky_to(a3[:].rearrange("c (r a b) -> c r a b", r=6, a=7), psum)
                    col = g * 12 + hh * 6
                    nc.vector.tensor_reduce(out=h_t[m][:, col:col + 6],
                                            in_=a3[:].rearrange("p (r q) -> p r q", r=6),
                                            axis=mybir.AxisListType.X, op=mybir.AluOpType.max)

                conv_layer(lambda kk, hh: (act2[kk], GUARD + hh * 6 * 81),
                           qw3_t, lambda kk, t, m: (kk * 9 + t) * 256 + 128 * m, 512, dst3, 2)

            bi = 0
            for g in range(NG):
                while bi * 8 < (g + 1) * 12 and bi < NB:
                    emit_pool_batch(bi)
                    bi += 1
                emit_conv_group(g)
            while bi < NB:
                emit_pool_batch(bi)
                bi += 1

            fc_ps = ps_c.tile([4, NROI], f32, tag="cm0")
            nc.tensor.matmul(fc_ps[:], lhsT=qfc_t[:, 0:4], rhs=h_t[0][:], start=True, stop=False)
            nc.tensor.matmul(fc_ps[:], lhsT=qfc_t[:, 4:8], rhs=h_t[1][:], start=False, stop=True)
            lg_sb = pool.tile([4, NROI], f32, tag="lg")
            nc.vector.tensor_tensor(out=lg_sb[:], in0=fc_ps[:],
                                    in1=qfcb_t[:].to_broadcast([4, NROI]),
                                    op=mybir.AluOpType.add)
            nc.sync.dma_start(lg_out[:], lg_sb[:])

    nc.finalize()
    _fix_multiwait(nc)
    return nc


# --------------------------------------------------------------------------
# host-side kernel orchestration
# --------------------------------------------------------------------------
_cache = {}


def _get_stage_a():
    if "a" not in _cache:
        _cache["a"] = _build_stage_a()
    return _cache["a"]


def _get_stage_b():
    if "b" not in _cache:
        _cache["b"] = _build_stage_b(f32r)
    return _cache["b"]


def _pack_conv_weights(inputs):
    qw1 = inputs["qw1"]  # [256, 64, 3, 3]
    qw2 = inputs["qw2"]
    qw3 = inputs["qw3"]
    qw1p = np.zeros((64, 9 * 256), dtype=np.float32)
    for t in range(9):
        dy, dx = t // 3, t % 3
        qw1p[:, t * 256:(t + 1) * 256] = qw1[:, :, dy, dx].T
    def pack23(qw):
        out = np.zeros((128, 18 * 256), dtype=np.float32)
        for kk in range(2):
            for t in range(9):
                dy, dx = t // 3, t % 3
                out[:, (kk * 9 + t) * 256:(kk * 9 + t + 1) * 256] = qw[:, kk * 128:(kk + 1) * 128, dy, dx].T
        return out
    qw2p = pack23(qw2)
    qw3p = pack23(qw3)
    qfcp = np.ascontiguousarray(inputs["qfc_w"].T.reshape(2, 128, 4).transpose(1, 0, 2).reshape(128, 8))
    qbp = np.concatenate([inputs["qb1"], inputs["qb2"], inputs["qb3"]]).reshape(1, -1).astype(np.float32)
    qfcb = inputs["qfc_b"].reshape(4, 1).astype(np.float32)
    return qw1p, qw2p, qw3p, qfcp, qbp, qfcb


def kernel(**inputs):
    inputs = {k: np.asarray(v) for k, v in inputs.items()}
    xhat = inputs["xhat"].astype(np.float32)          # [B,1,H,W]
    feat_up = inputs["feat_up"].astype(np.float32)     # [B,64,H,W]

    # ---------- host float64 patch-score path ----------
    s64 = _host_score64(inputs["feat_down"], inputs["pw1"], inputs["pb1"], inputs["pw2"],
                        inputs["pb2"], inputs["pw3"], inputs["pb3"], inputs["pfc_w"], inputs["pfc_b"])
    bad = s64 < 0.5                                    # [B,H,W] nominal mask
    uncertain = np.abs(s64 - 0.5) < SCORE_BAND
    xh = xhat[:, 0]                                    # [B,H,W]
    m_full = np.where(bad, np.float32(NEG), xh).astype(np.float32)

    nbad = max(int(bad.sum()), 1)
    mean_bad = float(np.where(bad, xh.astype(np.float64), 0.0).sum() / nbad)

    # ---------- stage A ----------
    nc_a = _get_stage_a()
    in_maps_a = []
    for c in range(N_CORES):
        b, half = c // 2, c % 2
        r0 = half * 256
        slab = np.full((SLAB, W), NEG, dtype=np.float32)
        lo, hi = r0 - 3, r0 + 259
        slo, shi = max(lo, 0), min(hi, H)
        slab[slo - lo:shi - lo, :] = m_full[b, slo:shi, :]
        idx = ((np.arange(r0, r0 + 256)[:, None] * W) + np.arange(W)[None, :]).astype(np.float32)
        in_maps_a.append({"m_in": slab, "idx_in": idx})
    res_a = run_bass_kernel_spmd(nc_a, in_maps_a, core_ids=list(range(N_CORES)))

    # ---------- host: merge candidates, top-k ----------
    need_fallback = False
    sel_idx = np.zeros((B, K), dtype=np.int64)
    sel_val = np.zeros((B, K), dtype=np.float32)
    for b in range(B):
        vals = np.concatenate([res_a.results[2 * b + h]["bvals"].reshape(-1) for h in range(2)])
        idxs = np.concatenate([res_a.results[2 * b + h]["bidx"].reshape(-1) for h in range(2)])
        keep = vals > NMS_THR
        v = vals[keep]
        ix = idxs[keep].astype(np.int64)
        if v.size < K:
            need_fallback = True
            break
        order = np.lexsort((ix, -v))
        v = v[order][:K + 32]
        ix = ix[order][:K + 32]
        sel_val[b] = v[:K]
        sel_idx[b] = ix[:K]
        # guards
        n_unc = int(uncertain[b].sum())
        ci = min(K + 8 * max(n_unc, 1), v.size - 1)
        cutoff_cons = v[ci] if v.size > K else v[K - 1]
        if not (v[K - 1] > mean_bad + 1e-3):
            need_fallback = True
            break
        if n_unc:
            uy, ux = np.nonzero(uncertain[b])
            if (xh[b, uy, ux] >= cutoff_cons - 1e-6).any():
                need_fallback = True
                break

    if need_fallback:
        return _full_reference_fallback(inputs)

    ys_all = (sel_idx // W).astype(np.int32)
    xs_all = (sel_idx % W).astype(np.int32)

    # ---------- stage B prep ----------
    nc_b = _get_stage_b()
    Mp = _pool_matrix()
    qw1p, qw2p, qw3p, qfcp, qbp, qfcb = _pack_conv_weights(inputs)
    in_maps_b = []
    for c in range(N_CORES):
        b, half = c // 2, c % 2
        ys = ys_all[b, half * 256:(half + 1) * 256]
        xs = xs_all[b, half * 256:(half + 1) * 256]
        patches = np.zeros((64, NROI * 121), dtype=np.float32)
        fb = feat_up[b]
        for r in range(256):
            y, x = int(ys[r]), int(xs[r])
            if 5 <= y <= H - 6 and 5 <= x <= W - 6:
                patch = fb[:, y - 5:y + 6, x - 5:x + 6]
            else:
                rows = np.clip(np.arange(y - 5, y + 6), 0, H - 1)
                cols = np.clip(np.arange(x - 5, x + 6), 0, W - 1)
                patch = fb[:, rows[:, None], cols[None, :]]   # [64,11,11]
            patches[:, r * 121:(r + 1) * 121] = patch.reshape(64, 121)
        in_maps_b.append({"patches": patches,
                          "mpool": Mp, "qw1p": qw1p, "qw2p": qw2p, "qw3p": qw3p,
                          "qfcp": qfcp, "qbp": qbp, "qfcb": qfcb})
    res_b = run_bass_kernel_spmd(nc_b, in_maps_b, core_ids=list(range(N_CORES)))

    # ---------- assemble outputs ----------
    cls_logits = np.zeros((B, K, NCLS), dtype=np.float32)
    for c in range(N_CORES):
        b, half = c // 2, c % 2
        lg = res_b.results[c]["lg"]            # [4, NROI]
        cls_logits[b, half * 256:(half + 1) * 256] = lg[:, :256].T
    coords = np.stack([xs_all, ys_all], axis=-1).astype(np.int32)
    top_s = sel_val.astype(np.float32)
    valid = top_s > NMS_THR
    return cls_logits, coords, top_s, valid


# revision 30
# speedup vs baseline: 1.6487x; 1.6487x over previous
"""Trainium2 Bass kernel for nn_CellDetectorWithClassifierInd.

Pipeline (B=4 images, 8 NeuronCores, data-parallel over half-images / roi halves):
  host   : patch-head MLP + softmax + bilinear resize in float64 (tiny: 1.6 GFLOP)
           -> bad mask + uncertainty band (needed to make the discrete mask
              decisions verifiable against the fp32 reference).
  stage A: (device) belief-map NMS: 7x7 separable max-pool, peak candidates,
           4x4 block max + argmax-index reduction.  Core c handles half of
           image c//2 (256 rows + 3-row halo).
  host   : merge candidates, exact top-k with lax.top_k tie-breaking, safety
           guards (falls back to a full jax reference recompute if any
           ambiguity could flip the selected peak set).
  stage B: (device) roi-align gather (dynamic-offset DMA from C-major feature
           map, 11x11 patches), bilinear pooling as matmul, 3x 3x3-conv head
           (float32r matmuls on the PE), spatial max, final FC.
  host   : assemble outputs.
"""

import numpy as np
import concourse.bass as bass
import concourse.mybir as mybir
from concourse.tile import TileContext
from concourse.bass_utils import run_bass_kernel_spmd
from concourse.masks import make_identity

f32 = mybir.dt.float32
f32r = mybir.dt.float32r
i32 = mybir.dt.int32

B, H, W = 4, 512, 512
DH = DW = 32
K = 512
NMS_THR = 0.2
MIN_DIST = 3
PROP_HALF = 5
POOL = 7
NCLS = 4
C_UP = 64

NEG = -1.0e38
GATE = 2.0e38

N_CORES = 8
SLAB = 262          # 256 rows + 3 halo each side
NROI = 264          # 256 real rois + 8 dummy (divisible by 8 and 12)
FEAT_ROWS = 528     # 512 image rows + 16 appendix rows for border-roi patches
APP_STRIDE = 16     # appendix packing stride (collision-free: |16*dk + dc| < 512)
MAX_BORDER = 31

SCORE_BAND = 2.0e-6  # |score64 - 0.5| band treated as uncertain (measured
                     # |score64 - score_ref_fp32| max ~2.7e-7, 7x margin)


# --------------------------------------------------------------------------
# BIR post-pass: split instructions with >1 semaphore waits (TRN2 walrus
# CoreV3 rejects multi-wait Drains emitted by the Tile kernel-tail barrier).
# --------------------------------------------------------------------------
def _fix_multiwait(nc, maxw=1):
    for bb in nc.m.functions[0].blocks:
        newlist = []
        for ins in bb.instructions:
            si = ins.sync_info
            if si is not None and si.on_wait is not None and len(si.on_wait) > maxw:
                waits = list(si.on_wait)
                for w in waits[maxw:]:
                    nop = mybir.InstNoOp(name=nc.get_next_instruction_name(), ins=[], outs=[])
                    nop.engine = ins.engine
                    nop.sync_info = mybir.SyncInfo(on_wait=[w], on_update=[])
                    newlist.append(nop)
                si.on_wait = waits[:maxw]
            newlist.append(ins)
        bb.instructions = newlist


# --------------------------------------------------------------------------
# Host: float64 patch-score path
# --------------------------------------------------------------------------
def _resize_matrix_1d(n_out, n_in):
    """Row-normalized triangle-kernel weight matrix matching
    jax.image.resize(..., method='bilinear') with align_corners=False."""
    scale = n_in / n_out
    i = np.arange(n_out, dtype=np.float64)
    x = (i + 0.5) * scale - 0.5          # sample position in input coords
    j = np.arange(n_in, dtype=np.float64)
    w = np.maximum(0.0, 1.0 - np.abs(x[:, None] - j[None, :]))
    s = w.sum(axis=1, keepdims=True)
    return w / s


def _host_score64(feat_down, pw1, pb1, pw2, pb2, pw3, pb3, pfc_w, pfc_b):
    X = feat_down.reshape(B, 256, DH * DW).astype(np.float64)

    def lk(v):
        return np.where(v > 0, v, 0.1 * v)

    f = lk(np.matmul(pw1.astype(np.float64), X) + pb1.astype(np.float64)[None, :, None])
    f = lk(np.matmul(pw2.astype(np.float64), f) + pb2.astype(np.float64)[None, :, None])
    f = lk(np.matmul(pw3.astype(np.float64), f) + pb3.astype(np.float64)[None, :, None])
    pl = np.matmul(pfc_w.astype(np.float64), f) + pfc_b.astype(np.float64)[None, :, None]
    d = pl[:, 1] - pl[:, 0]
    score32 = (1.0 / (1.0 + np.exp(-d))).reshape(B, DH, DW)
    Wm = _resize_matrix_1d(H, DH)        # [512, 32]
    s512 = np.einsum("yi,bij,xj->byx", Wm, score32, Wm, optimize=True)
    return s512                           # [B, 512, 512] float64


# --------------------------------------------------------------------------
# Host: full reference recompute in jax (CPU) -- correctness fallback
# --------------------------------------------------------------------------
def _full_reference_fallback(inputs):
    import jax
    import jax.numpy as jnp
    from jax import lax

    cpu = jax.devices("cpu")[0]
    with jax.default_device(cpu):
        xhat = jnp.asarray(inputs["xhat"])
        feat_down = jnp.asarray(inputs["feat_down"])
        feat_up = jnp.asarray(inputs["feat_up"])

        def leaky(x):
            return jnp.where(x > 0, x, 0.1 * x)

        f = leaky(jnp.einsum('bchw,oc->bohw', feat_down, inputs["pw1"]) + inputs["pb1"][None, :, None, None])
        f = leaky(jnp.einsum('bchw,oc->bohw', f, inputs["pw2"]) + inputs["pb2"][None, :, None, None])
        f = leaky(jnp.einsum('bchw,oc->bohw', f, inputs["pw3"]) + inputs["pb3"][None, :, None, None])
        plogits = jnp.einsum('bchw,oc->bohw', f, inputs["pfc_w"]) + inputs["pfc_b"][None, :, None, None]
        patch_score = jax.nn.softmax(plogits, axis=1)[:, 1:2]
        patch_score = jax.image.resize(patch_score, (B, 1, H, W), 'bilinear')
        bad = patch_score < 0.5
        nbad = jnp.maximum(jnp.sum(bad), 1).astype(xhat.dtype)
        mean_bad = jnp.sum(jnp.where(bad, xhat, 0.0)) / nbad
        bm = jnp.where(bad, mean_bad, xhat)[:, 0]
        win = 2 * MIN_DIST + 1
        mp = lax.reduce_window(bm, -jnp.inf, lax.max, (1, win, win), (1, 1, 1), 'SAME')
        peak_scores = jnp.where((bm == mp) & (bm > NMS_THR), bm, -jnp.inf)
        top_s, top_i = lax.top_k(peak_scores.reshape(B, -1), K)
        ys = (top_i // W).astype(jnp.int32)
        xs = (top_i % W).astype(jnp.int32)
        valid = top_s > NMS_THR
        coords = jnp.stack([xs, ys], axis=-1)
        roi_size = 2.0 * PROP_HALF
        offs = (jnp.arange(POOL, dtype=jnp.float32) + 0.5) * (roi_size / POOL)
        px = (xs.astype(jnp.float32) - PROP_HALF)[..., None] + offs
        py = (ys.astype(jnp.float32) - PROP_HALF)[..., None] + offs

        def sample_one(feat, py_i, px_i):
            yy = jnp.broadcast_to(py_i[:, :, None], (K, POOL, POOL))
            xx = jnp.broadcast_to(px_i[:, None, :], (K, POOL, POOL))
            y0 = jnp.floor(yy); x0 = jnp.floor(xx)
            wy = yy - y0; wx = xx - x0
            y0i = jnp.clip(y0.astype(jnp.int32), 0, H - 1)
            y1i = jnp.clip(y0.astype(jnp.int32) + 1, 0, H - 1)
            x0i = jnp.clip(x0.astype(jnp.int32), 0, W - 1)
            x1i = jnp.clip(x0.astype(jnp.int32) + 1, 0, W - 1)
            g = lambda yi, xi: feat[:, yi, xi]
            return (g(y0i, x0i) * (1 - wy) * (1 - wx) + g(y0i, x1i) * (1 - wy) * wx
                    + g(y1i, x0i) * wy * (1 - wx) + g(y1i, x1i) * wy * wx)

        pooled = jax.vmap(sample_one)(feat_up, py, px)
        pooled = pooled.transpose(0, 2, 1, 3, 4).reshape(B * K, C_UP, POOL, POOL)
        dn = ('NCHW', 'OIHW', 'NCHW')
        from jax import lax as _lax
        h = leaky(_lax.conv_general_dilated(pooled, inputs["qw1"], (1, 1), 'SAME', dimension_numbers=dn) + inputs["qb1"][None, :, None, None])
        h = leaky(_lax.conv_general_dilated(h, inputs["qw2"], (1, 1), 'SAME', dimension_numbers=dn) + inputs["qb2"][None, :, None, None])
        h = leaky(_lax.conv_general_dilated(h, inputs["qw3"], (1, 1), 'SAME', dimension_numbers=dn) + inputs["qb3"][None, :, None, None])
        h = jnp.max(h, axis=(2, 3))
        cls_logits = (h @ inputs["qfc_w"].T + inputs["qfc_b"]).reshape(B, K, NCLS)
        return (np.asarray(cls_logits), np.asarray(coords), np.asarray(top_s), np.asarray(valid))


# --------------------------------------------------------------------------
# Stage A device program: NMS + candidate block-reduction
# --------------------------------------------------------------------------
def _build_stage_a():
    nc = bass.Bass()
    m_in = nc.dram_tensor("m_in", [SLAB, W], f32, kind="ExternalInput")
    idx_in = nc.dram_tensor("idx_in", [256, W], f32, kind="ExternalInput")
    bvals = nc.dram_tensor("bvals", [64, 128], f32, kind="ExternalOutput")
    bidx = nc.dram_tensor("bidx", [64, 128], f32, kind="ExternalOutput")
    mx_d = nc.dram_tensor("mx_d", [SLAB, W], f32)
    cand_d = nc.dram_tensor("cand_d", [256, W], f32)

    with TileContext(nc) as tc:
        with tc.tile_pool(name="sb", bufs=2) as pool, \
             tc.tile_pool(name="sb1", bufs=1) as pool1:
            # ---- phase 1: x-direction 7-window max
            for c0, rows in ((0, 128), (128, 128), (256, 6)):
                pad = pool.tile([128, W + 6], f32, tag="pad")
                nc.vector.memset(pad[:rows, :], NEG)
                nc.sync.dma_start(pad[:rows, 3:W + 3], m_in[c0:c0 + rows, :])
                mx = pool.tile([128, W], f32, tag="mx")
                nc.vector.tensor_max(mx[:rows, :], pad[:rows, 0:W], pad[:rows, 1:W + 1])
                for d in range(2, 7):
                    nc.vector.tensor_max(mx[:rows, :], mx[:rows, :], pad[:rows, d:W + d])
                nc.sync.dma_start(mx_d[c0:c0 + rows, :], mx[:rows, :])
            # ---- phase 2: y-direction 7-window max + candidate mask
            for j0, rows in ((0, 120), (120, 120), (240, 16)):
                acc = pool.tile([128, W], f32, tag="yacc")
                nc.sync.dma_start(acc[:rows, :], mx_d[j0:j0 + rows, :])
                for k in range(1, 7):
                    tk = pool.tile([128, W], f32, tag="yld")
                    nc.sync.dma_start(tk[:rows, :], mx_d[j0 + k:j0 + k + rows, :])
                    nc.vector.tensor_max(acc[:rows, :], acc[:rows, :], tk[:rows, :])
                m_t = pool.tile([128, W], f32, tag="mrow")
                nc.sync.dma_start(m_t[:rows, :], m_in[j0 + 3:j0 + 3 + rows, :])
                e01 = pool.tile([128, W], f32, tag="e01")
                nc.vector.tensor_tensor(out=e01[:rows, :], in0=m_t[:rows, :], in1=acc[:rows, :],
                                        op=mybir.AluOpType.is_equal)
                g01 = pool.tile([128, W], f32, tag="g01")
                nc.vector.tensor_scalar(out=g01[:rows, :], in0=m_t[:rows, :], scalar1=float(NMS_THR),
                                        scalar2=None, op0=mybir.AluOpType.is_gt)
                c5 = pool.tile([128, W], f32, tag="c5")
                nc.vector.tensor_tensor(out=c5[:rows, :], in0=e01[:rows, :], in1=g01[:rows, :],
                                        op=mybir.AluOpType.mult)
                nc.vector.tensor_scalar(out=c5[:rows, :], in0=c5[:rows, :], scalar1=0.5, scalar2=None,
                                        op0=mybir.AluOpType.subtract)
                cand = pool.tile([128, W], f32, tag="cand")
                nc.vector.scalar_tensor_tensor(out=cand[:rows, :], in0=c5[:rows, :], scalar=GATE,
                                               in1=m_t[:rows, :], op0=mybir.AluOpType.mult,
                                               op1=mybir.AluOpType.min)
                nc.sync.dma_start(cand_d[j0:j0 + rows, :], cand[:rows, :])
            # ---- phase 3: 4x4 block max + argmax index
            bl = pool1.tile([64, 2048], f32, tag="bl")
            il = pool1.tile([64, 2048], f32, tag="il")
            for rr in range(4):
                src = cand_d.ap().rearrange("(q four) n -> q four n", four=4)[:, rr, :]
                nc.sync.dma_start(bl[:, rr * 512:(rr + 1) * 512].rearrange("p (a c) -> p a c", c=4),
                                  src.rearrange("q (a c) -> q a c", c=4))
                srci = idx_in.ap().rearrange("(q four) n -> q four n", four=4)[:, rr, :]
                nc.sync.dma_start(il[:, rr * 512:(rr + 1) * 512].rearrange("p (a c) -> p a c", c=4),
                                  srci.rearrange("q (a c) -> q a c", c=4))
            bl4 = bl[:].rearrange("p (b a c) -> p a b c", b=4, c=4)
            bmax = pool1.tile([64, 128], f32, tag="bmax")
            nc.vector.tensor_reduce(out=bmax[:], in_=bl4, axis=mybir.AxisListType.XY,
                                    op=mybir.AluOpType.max)
            nc.sync.dma_start(bvals[:], bmax[:])
            eqt = pool1.tile([64, 2048], f32, tag="eqt")
            eq4 = eqt[:].rearrange("p (b a c) -> p a b c", b=4, c=4)
            bb = bmax[:].to_broadcast([64, 128, 4, 4])
            nc.vector.tensor_tensor(out=eq4, in0=bl4, in1=bb, op=mybir.AluOpType.is_equal)
            nc.vector.tensor_scalar(out=eqt[:], in0=eqt[:], scalar1=0.5, scalar2=None,
                                    op0=mybir.AluOpType.subtract)
            isel = pool1.tile([64, 2048], f32, tag="isel")
            nc.vector.scalar_tensor_tensor(out=isel[:], in0=eqt[:], scalar=-GATE,
                                           in1=il[:], op0=mybir.AluOpType.mult,
                                           op1=mybir.AluOpType.max)
            bai = pool1.tile([64, 128], f32, tag="bai")
            nc.vector.tensor_reduce(out=bai[:], in_=isel[:].rearrange("p (b a c) -> p a b c", b=4, c=4),
                                    axis=mybir.AxisListType.XY, op=mybir.AluOpType.min)
            nc.sync.dma_start(bidx[:], bai[:])

    nc.finalize()
    _fix_multiwait(nc)
    return nc


# --------------------------------------------------------------------------
# Stage B device program: roi gather + pooling + conv head
# --------------------------------------------------------------------------
def _pool_matrix():
    """Shared [121, 49] bilinear pooling matrix (canonical fractional weights)."""
    offs = (np.arange(POOL, dtype=np.float32) + np.float32(0.5)) * np.float32(2.0 * PROP_HALF / POOL)
    u0 = np.floor(offs).astype(np.int64)
    fr = (offs - u0.astype(np.float32)).astype(np.float32)
    M = np.zeros((121, 49), dtype=np.float32)
    for i in range(POOL):
        for j in range(POOL):
            s = i * 7 + j
            for du, wu in ((0, np.float32(1.0) - fr[i]), (1, fr[i])):
                for dv, wv in ((0, np.float32(1.0) - fr[j]), (1, fr[j])):
                    t = (u0[i] + du) * 11 + (u0[j] + dv)
                    M[t, s] += wu * wv
    return M


def _build_stage_b(dt_conv=f32r):
    nc = bass.Bass()
    patches = nc.dram_tensor("patches", [64, NROI * 121], f32, kind="ExternalInput")
    mpool_in = nc.dram_tensor("mpool", [121, 49], f32, kind="ExternalInput")
    qw1_in = nc.dram_tensor("qw1p", [64, 9 * 256], f32, kind="ExternalInput")
    qw2_in = nc.dram_tensor("qw2p", [128, 18 * 256], f32, kind="ExternalInput")
    qw3_in = nc.dram_tensor("qw3p", [128, 18 * 256], f32, kind="ExternalInput")
    qfc_in = nc.dram_tensor("qfcp", [128, 8], f32, kind="ExternalInput")
    qb_in = nc.dram_tensor("qbp", [1, 3 * 256], f32, kind="ExternalInput")   # qb1|qb2|qb3
    qfcb_in = nc.dram_tensor("qfcb", [4, 1], f32, kind="ExternalInput")
    lg_out = nc.dram_tensor("lg", [4, NROI], f32, kind="ExternalOutput")

    NB = NROI // 8     # pooling batches
    NG = NROI // 12    # conv groups

    with TileContext(nc) as tc:
        with tc.tile_pool(name="cst", bufs=1) as cpool, \
             tc.tile_pool(name="wrk", bufs=3) as pool, \
             tc.tile_pool(name="act", bufs=2) as apool, \
             tc.tile_pool(name="ps_t", bufs=1, space="PSUM") as ps_t, \
             tc.tile_pool(name="ps_p", bufs=1, space="PSUM") as ps_p, \
             tc.tile_pool(name="ps_r", bufs=1, space="PSUM") as ps_r, \
             tc.tile_pool(name="ps_c", bufs=2, space="PSUM") as ps_c:
            ident = cpool.tile([128, 128], f32)
            make_identity(nc, ident)
            mp_t = cpool.tile([121, 49], f32, tag="mp")
            nc.sync.dma_start(mp_t[:], mpool_in[:])
            qw1_t = cpool.tile([64, 9 * 256], dt_conv, tag="qw1")
            nc.gpsimd.dma_start(qw1_t[:], qw1_in[:])
            qw2_t = cpool.tile([128, 18 * 256], dt_conv, tag="qw2")
            nc.gpsimd.dma_start(qw2_t[:], qw2_in[:])
            qw3_t = cpool.tile([128, 18 * 256], dt_conv, tag="qw3")
            nc.gpsimd.dma_start(qw3_t[:], qw3_in[:])
            qfc_t = cpool.tile([128, 8], dt_conv, tag="qfc")
            nc.gpsimd.dma_start(qfc_t[:], qfc_in[:])
            qb_t = cpool.tile([1, 3 * 256], dt_conv, tag="qb")
            nc.gpsimd.dma_start(qb_t[:], qb_in[:])
            qfcb_t = cpool.tile([4, 1], f32, tag="qfcb")
            nc.sync.dma_start(qfcb_t[:], qfcb_in[:])
            ones_t = cpool.tile([1, 486], dt_conv, tag="ones")
            nc.vector.memset(ones_t[:].bitcast(f32), 1.0)


            # GUARD elements on each side so tap-shifted contiguous reads stay in-tile
            GUARD = 16
            pooled = cpool.tile([64, GUARD + NROI * 81 + GUARD], dt_conv, tag="pooled")
            nc.vector.memset(pooled[:].bitcast(f32), 0.0)
            h0 = cpool.tile([128, NROI], dt_conv, tag="h0")
            h1 = cpool.tile([128, NROI], dt_conv, tag="h1")
            h_t = (h0, h1)

            def emit_pool_batch(bi):
                batch = pool.tile([121, 512], f32, tag="batch")
                pt8 = pool.tile([64, 8 * 121], f32, tag="pt8")
                nc.sync.dma_start(pt8[:], patches[:, bi * 8 * 121:(bi + 1) * 8 * 121])
                for s in range(8):
                    pt_ps = ps_t.tile([121, 64], f32, tag="tp")
                    nc.tensor.transpose(pt_ps[:], pt8[:, s * 121:(s + 1) * 121], ident[:64, :64])
                    nc.vector.tensor_copy(batch[:, s * 64:(s + 1) * 64], pt_ps[:])
                pl_ps = ps_p.tile([49, 512], f32, tag="pp")
                nc.tensor.matmul(pl_ps[:], lhsT=mp_t[:], rhs=batch[:], start=True, stop=True)
                pl_sb = pool.tile([49, 512], f32, tag="plsb")
                nc.vector.tensor_copy(pl_sb[:], pl_ps[:])
                for s in range(8):
                    r = bi * 8 + s
                    rt_ps = ps_r.tile([64, 49], f32, tag="rp")
                    nc.tensor.transpose(rt_ps[:], pl_sb[:, s * 64:(s + 1) * 64], ident[:49, :49])
                    base = GUARD + r * 81 + 10
                    dst = pooled[:, base:base + 63].rearrange("c (r w) -> c r w", r=7, w=9)[:, :, :7]
                    nc.vector.tensor_copy(dst, rt_ps[:].rearrange("c (r w) -> c r w", r=7, w=7))

            # Conv layers compute the FULL padded 9x9 grid per roi (N = 6*81 = 486
            # per matmul): every tap reads a CONTIGUOUS 486-window at a +-10
            # shifted offset (fp32r matmuls reject non-contiguous moving APs).
            # Ring outputs accumulate garbage and are simply never read.
            VIEW49 = ("c (r a b) -> c r a b", dict(r=6, a=9))

            def valid49(tile_ap, elem_base):
                """[P, 6, 7, 7] view of the 49 valid positions of 6 rois' 81-maps."""
                v = tile_ap[:, elem_base:elem_base + 6 * 81].rearrange("c (r a b) -> c r a b", r=6, a=9)
                return v[:, :, 1:8, 1:8]

            def conv_layer(src_of, wt, wcols_fn, bias_col0, dst_fn, kchunks):
                # one conv layer for a 12-roi group processed as 2 halves of 6
                for hh in range(2):
                    for m in range(2):
                        psum = ps_c.tile([128, 486], f32, tag=f"cm{m}")
                        nc.tensor.matmul(psum[:], lhsT=qb_t[0:1, bias_col0 + 128 * m:bias_col0 + 128 * m + 128],
                                         rhs=ones_t[:], start=True, stop=False)
                        n_mm = kchunks * 9
                        cnt = 0
                        for kk in range(kchunks):
                            tile_ap, base = src_of(kk, hh)    # base = element idx of roi0's 81-map
                            for t in range(9):
                                dy, dx = t // 3, t % 3
                                off = base + 9 * (dy - 1) + (dx - 1)
                                rhs = tile_ap[:, off:off + 486]
                                cnt += 1
                                nc.tensor.matmul(psum[:], lhsT=wt[:, wcols_fn(kk, t, m):wcols_fn(kk, t, m) + 128],
                                                 rhs=rhs, start=False, stop=(cnt == n_mm))
                        dst_fn(hh, m, psum)

            def leaky_to(dst49, psum):
                """dst49: [128, 6, 7, 7] SBUF view. psum: [128, 486] full grid."""
                pv = psum[:].rearrange("c (r a b) -> c r a b", r=6, a=9)[:, :, 1:8, 1:8]
                tmp = pool.tile([128, 294], f32, tag="lk", name=f"lk{nc.next_id()}")
                t4 = tmp[:].rearrange("c (r a b) -> c r a b", r=6, a=7)
                nc.scalar.mul(t4, pv, 0.1)
                nc.vector.tensor_tensor(out=dst49, in0=t4, in1=pv, op=mybir.AluOpType.max)

            ACT_GSZ = GUARD + 12 * 81 + GUARD

            def emit_conv_group(g):
                # conv1: input = pooled rois [12g .. 12g+11]
                act1 = [apool.tile([128, ACT_GSZ], dt_conv, name=f"act1_{g}_{m}", tag=f"a1k{m}") for m in range(2)]
                act2 = [apool.tile([128, ACT_GSZ], dt_conv, name=f"act2_{g}_{m}", tag=f"a2k{m}") for m in range(2)]
                for m in range(2):
                    nc.vector.memset(act1[m][:].bitcast(f32), 0.0)
                    nc.vector.memset(act2[m][:].bitcast(f32), 0.0)

                def dst1(hh, m, psum):
                    leaky_to(valid49(act1[m], GUARD + hh * 6 * 81), psum)

                conv_layer(lambda kk, hh: (pooled, GUARD + (g * 12 + hh * 6) * 81),
                           qw1_t, lambda kk, t, m: t * 256 + 128 * m, 0, dst1, 1)

                def dst2(hh, m, psum):
                    leaky_to(valid49(act2[m], GUARD + hh * 6 * 81), psum)

                conv_layer(lambda kk, hh: (act1[kk], GUARD + hh * 6 * 81),
                           qw2_t, lambda kk, t, m: (kk * 9 + t) * 256 + 128 * m, 256, dst2, 2)

                # conv3 + leaky + spatial max -> h
                def dst3(hh, m, psum):
                    a3 = pool.tile([128, 294], dt_conv, tag="a3")
                    leaky_to(a3[:].rearrange("c (r a b) -> c r a b", r=6, a=7), psum)
                    col = g * 12 + hh * 6
                    nc.vector.tensor_reduce(out=h_t[m][:, col:col + 6],
                                            in_=a3[:].rearrange("p (r q) -> p r q", r=6),
                                            axis=mybir.AxisListType.X, op=mybir.AluOpType.max)

                conv_layer(lambda kk, hh: (act2[kk], GUARD + hh * 6 * 81),
                           qw3_t, lambda kk, t, m: (kk * 9 + t) * 256 + 128 * m, 512, dst3, 2)

            bi = 0
            for g in range(NG):
                while bi * 8 < (g + 1) * 12 and bi < NB:
                    emit_pool_batch(bi)
                    bi += 1
                emit_conv_group(g)
            while bi < NB:
                emit_pool_batch(bi)
                bi += 1

            fc_ps = ps_c.tile([4, NROI], f32, tag="cm0")
            nc.tensor.matmul(fc_ps[:], lhsT=qfc_t[:, 0:4], rhs=h_t[0][:], start=True, stop=False)
            nc.tensor.matmul(fc_ps[:], lhsT=qfc_t[:, 4:8], rhs=h_t[1][:], start=False, stop=True)
            lg_sb = pool.tile([4, NROI], f32, tag="lg")
            nc.vector.tensor_tensor(out=lg_sb[:], in0=fc_ps[:],
                                    in1=qfcb_t[:].to_broadcast([4, NROI]),
                                    op=mybir.AluOpType.add)
            nc.sync.dma_start(lg_out[:], lg_sb[:])

    nc.finalize()
    _fix_multiwait(nc)
    return nc


# --------------------------------------------------------------------------
# host-side kernel orchestration
# --------------------------------------------------------------------------
_cache = {}
last_timings = {}
last_in_maps = {}


def _get_stage_a():
    if "a" not in _cache:
        _cache["a"] = _build_stage_a()
    return _cache["a"]


def _get_stage_b():
    if "b" not in _cache:
        _cache["b"] = _build_stage_b(f32r)
    return _cache["b"]


def _pack_conv_weights(inputs):
    qw1 = inputs["qw1"]  # [256, 64, 3, 3]
    qw2 = inputs["qw2"]
    qw3 = inputs["qw3"]
    qw1p = np.zeros((64, 9 * 256), dtype=np.float32)
    for t in range(9):
        dy, dx = t // 3, t % 3
        qw1p[:, t * 256:(t + 1) * 256] = qw1[:, :, dy, dx].T
    def pack23(qw):
        out = np.zeros((128, 18 * 256), dtype=np.float32)
        for kk in range(2):
            for t in range(9):
                dy, dx = t // 3, t % 3
                out[:, (kk * 9 + t) * 256:(kk * 9 + t + 1) * 256] = qw[:, kk * 128:(kk + 1) * 128, dy, dx].T
        return out
    qw2p = pack23(qw2)
    qw3p = pack23(qw3)
    qfcp = np.ascontiguousarray(inputs["qfc_w"].T.reshape(2, 128, 4).transpose(1, 0, 2).reshape(128, 8))
    qbp = np.concatenate([inputs["qb1"], inputs["qb2"], inputs["qb3"]]).reshape(1, -1).astype(np.float32)
    qfcb = inputs["qfc_b"].reshape(4, 1).astype(np.float32)
    return qw1p, qw2p, qw3p, qfcp, qbp, qfcb


def kernel(**inputs):
    inputs = {k: np.asarray(v) for k, v in inputs.items()}
    xhat = inputs["xhat"].astype(np.float32)          # [B,1,H,W]
    feat_up = inputs["feat_up"].astype(np.float32)     # [B,64,H,W]

    # ---------- host float64 patch-score path ----------
    s64 = _host_score64(inputs["feat_down"], inputs["pw1"], inputs["pb1"], inputs["pw2"],
                        inputs["pb2"], inputs["pw3"], inputs["pb3"], inputs["pfc_w"], inputs["pfc_b"])
    bad = s64 < 0.5                                    # [B,H,W] nominal mask
    uncertain = np.abs(s64 - 0.5) < SCORE_BAND
    xh = xhat[:, 0]                                    # [B,H,W]
    m_full = np.where(bad, np.float32(NEG), xh).astype(np.float32)

    nbad = max(int(bad.sum()), 1)
    mean_bad = float(np.where(bad, xh.astype(np.float64), 0.0).sum() / nbad)

    # ---------- stage A ----------
    nc_a = _get_stage_a()
    in_maps_a = []
    for c in range(N_CORES):
        b, half = c // 2, c % 2
        r0 = half * 256
        slab = np.full((SLAB, W), NEG, dtype=np.float32)
        lo, hi = r0 - 3, r0 + 259
        slo, shi = max(lo, 0), min(hi, H)
        slab[slo - lo:shi - lo, :] = m_full[b, slo:shi, :]
        idx = ((np.arange(r0, r0 + 256)[:, None] * W) + np.arange(W)[None, :]).astype(np.float32)
        in_maps_a.append({"m_in": slab, "idx_in": idx})
    import time as _time
    _t0 = _time.time()
    res_a = run_bass_kernel_spmd(nc_a, in_maps_a, core_ids=list(range(N_CORES)))
    last_timings["stage_a_wall"] = _time.time() - _t0
    last_in_maps["a"] = in_maps_a

    # ---------- host: merge candidates, top-k ----------
    need_fallback = False
    sel_idx = np.zeros((B, K), dtype=np.int64)
    sel_val = np.zeros((B, K), dtype=np.float32)
    for b in range(B):
        vals = np.concatenate([res_a.results[2 * b + h]["bvals"].reshape(-1) for h in range(2)])
        idxs = np.concatenate([res_a.results[2 * b + h]["bidx"].reshape(-1) for h in range(2)])
        keep = vals > NMS_THR
        v = vals[keep]
        ix = idxs[keep].astype(np.int64)
        if v.size < K:
            need_fallback = True
            break
        order = np.lexsort((ix, -v))
        v = v[order]
        ix = ix[order]
        sel_val[b] = v[:K]
        sel_idx[b] = ix[:K]
        # guards
        n_unc = int(uncertain[b].sum())
        ci = min(K + 8 * max(n_unc, 1), v.size - 1)
        cutoff_cons = v[ci]
        if not (v[K - 1] > mean_bad + 1e-3):
            need_fallback = True
            break
        if n_unc:
            uy, ux = np.nonzero(uncertain[b])
            if (xh[b, uy, ux] >= cutoff_cons - 1e-6).any():
                need_fallback = True
                break

    if need_fallback:
        return _full_reference_fallback(inputs)

    ys_all = (sel_idx // W).astype(np.int32)
    xs_all = (sel_idx % W).astype(np.int32)

    # ---------- stage B prep ----------
    nc_b = _get_stage_b()
    Mp = _pool_matrix()
    qw1p, qw2p, qw3p, qfcp, qbp, qfcb = _pack_conv_weights(inputs)
    in_maps_b = []
    for c in range(N_CORES):
        b, half = c // 2, c % 2
        ys = ys_all[b, half * 256:(half + 1) * 256]
        xs = xs_all[b, half * 256:(half + 1) * 256]
        patches = np.zeros((64, NROI * 121), dtype=np.float32)
        fb = feat_up[b]
        for r in range(256):
            y, x = int(ys[r]), int(xs[r])
            if 5 <= y <= H - 6 and 5 <= x <= W - 6:
                patch = fb[:, y - 5:y + 6, x - 5:x + 6]
            else:
                rows = np.clip(np.arange(y - 5, y + 6), 0, H - 1)
                cols = np.clip(np.arange(x - 5, x + 6), 0, W - 1)
                patch = fb[:, rows[:, None], cols[None, :]]   # [64,11,11]
            patches[:, r * 121:(r + 1) * 121] = patch.reshape(64, 121)
        in_maps_b.append({"patches": patches,
                          "mpool": Mp, "qw1p": qw1p, "qw2p": qw2p, "qw3p": qw3p,
                          "qfcp": qfcp, "qbp": qbp, "qfcb": qfcb})
    _t0 = _time.time()
    res_b = run_bass_kernel_spmd(nc_b, in_maps_b, core_ids=list(range(N_CORES)))
    last_timings["stage_b_wall"] = _time.time() - _t0
    last_in_maps["b"] = in_maps_b

    # ---------- assemble outputs ----------
    cls_logits = np.zeros((B, K, NCLS), dtype=np.float32)
    for c in range(N_CORES):
        b, half = c // 2, c % 2
        lg = res_b.results[c]["lg"]            # [4, NROI]
        cls_logits[b, half * 256:(half + 1) * 256] = lg[:, :256].T
    coords = np.stack([xs_all, ys_all], axis=-1).astype(np.int32)
    top_s = sel_val.astype(np.float32)
    valid = top_s > NMS_THR
    return cls_logits, coords, top_s, valid


# revision 32
# speedup vs baseline: 1.7929x; 1.0875x over previous
"""Trainium2 Bass kernel for nn_CellDetectorWithClassifierInd.

Pipeline (B=4 images, 8 NeuronCores, data-parallel over half-images / roi halves):
  host   : patch-head MLP + softmax + bilinear resize in float64 (tiny: 1.6 GFLOP)
           -> bad mask + uncertainty band (needed to make the discrete mask
              decisions verifiable against the fp32 reference).
  stage A: (device) belief-map NMS: 7x7 separable max-pool, peak candidates,
           4x4 block max + argmax-index reduction.  Core c handles half of
           image c//2 (256 rows + 3-row halo).
  host   : merge candidates, exact top-k with lax.top_k tie-breaking, safety
           guards (falls back to a full jax reference recompute if any
           ambiguity could flip the selected peak set).
  stage B: (device) roi-align bilinear pooling as a [121->49] matmul over
           host-sliced 11x11x64 patches, then the 3x 3x3-conv/leaky head as
           tap-shift-accumulated float32r matmuls on the PE (each tap reads a
           contiguous 486-wide window of a padded per-roi 9x9 grid; ring
           outputs are computed-and-ignored because fp32r matmuls reject
           non-contiguous moving APs), spatial max, final FC.
  host   : assemble outputs.
"""

import numpy as np
import concourse.bass as bass
import concourse.mybir as mybir
from concourse.tile import TileContext
from concourse.bass_utils import run_bass_kernel_spmd
from concourse.masks import make_identity

f32 = mybir.dt.float32
f32r = mybir.dt.float32r
i32 = mybir.dt.int32

B, H, W = 4, 512, 512
DH = DW = 32
K = 512
NMS_THR = 0.2
MIN_DIST = 3
PROP_HALF = 5
POOL = 7
NCLS = 4
C_UP = 64

NEG = -1.0e38
GATE = 2.0e38

N_CORES = 8
SLAB = 262          # 256 rows + 3 halo each side
NROI = 264          # 256 real rois + 8 dummy (divisible by 8 and 12)

SCORE_BAND = 2.0e-6  # |score64 - 0.5| band treated as uncertain (measured
                     # |score64 - score_ref_fp32| max ~2.7e-7, 7x margin)


# --------------------------------------------------------------------------
# BIR post-pass: split instructions with >1 semaphore waits (TRN2 walrus
# CoreV3 rejects multi-wait Drains emitted by the Tile kernel-tail barrier).
# --------------------------------------------------------------------------
def _fix_multiwait(nc, maxw=1):
    for bb in nc.m.functions[0].blocks:
        newlist = []
        for ins in bb.instructions:
            si = ins.sync_info
            if si is not None and si.on_wait is not None and len(si.on_wait) > maxw:
                waits = list(si.on_wait)
                for w in waits[maxw:]:
                    nop = mybir.InstNoOp(name=nc.get_next_instruction_name(), ins=[], outs=[])
                    nop.engine = ins.engine
                    nop.sync_info = mybir.SyncInfo(on_wait=[w], on_update=[])
                    newlist.append(nop)
                si.on_wait = waits[:maxw]
            newlist.append(ins)
        bb.instructions = newlist


# --------------------------------------------------------------------------
# Host: float64 patch-score path
# --------------------------------------------------------------------------
def _resize_matrix_1d(n_out, n_in):
    """Row-normalized triangle-kernel weight matrix matching
    jax.image.resize(..., method='bilinear') with align_corners=False."""
    scale = n_in / n_out
    i = np.arange(n_out, dtype=np.float64)
    x = (i + 0.5) * scale - 0.5          # sample position in input coords
    j = np.arange(n_in, dtype=np.float64)
    w = np.maximum(0.0, 1.0 - np.abs(x[:, None] - j[None, :]))
    s = w.sum(axis=1, keepdims=True)
    return w / s


def _host_score64(feat_down, pw1, pb1, pw2, pb2, pw3, pb3, pfc_w, pfc_b):
    X = feat_down.reshape(B, 256, DH * DW).astype(np.float64)

    def lk(v):
        return np.where(v > 0, v, 0.1 * v)

    f = lk(np.matmul(pw1.astype(np.float64), X) + pb1.astype(np.float64)[None, :, None])
    f = lk(np.matmul(pw2.astype(np.float64), f) + pb2.astype(np.float64)[None, :, None])
    f = lk(np.matmul(pw3.astype(np.float64), f) + pb3.astype(np.float64)[None, :, None])
    pl = np.matmul(pfc_w.astype(np.float64), f) + pfc_b.astype(np.float64)[None, :, None]
    d = pl[:, 1] - pl[:, 0]
    score32 = (1.0 / (1.0 + np.exp(-d))).reshape(B, DH, DW)
    Wm = _resize_matrix_1d(H, DH)        # [512, 32]
    s512 = np.einsum("yi,bij,xj->byx", Wm, score32, Wm, optimize=True)
    return s512                           # [B, 512, 512] float64


# --------------------------------------------------------------------------
# Host: full reference recompute in jax (CPU) -- correctness fallback
# --------------------------------------------------------------------------
def _full_reference_fallback(inputs):
    import jax
    import jax.numpy as jnp
    from jax import lax

    cpu = jax.devices("cpu")[0]
    with jax.default_device(cpu):
        xhat = jnp.asarray(inputs["xhat"])
        feat_down = jnp.asarray(inputs["feat_down"])
        feat_up = jnp.asarray(inputs["feat_up"])

        def leaky(x):
            return jnp.where(x > 0, x, 0.1 * x)

        f = leaky(jnp.einsum('bchw,oc->bohw', feat_down, inputs["pw1"]) + inputs["pb1"][None, :, None, None])
        f = leaky(jnp.einsum('bchw,oc->bohw', f, inputs["pw2"]) + inputs["pb2"][None, :, None, None])
        f = leaky(jnp.einsum('bchw,oc->bohw', f, inputs["pw3"]) + inputs["pb3"][None, :, None, None])
        plogits = jnp.einsum('bchw,oc->bohw', f, inputs["pfc_w"]) + inputs["pfc_b"][None, :, None, None]
        patch_score = jax.nn.softmax(plogits, axis=1)[:, 1:2]
        patch_score = jax.image.resize(patch_score, (B, 1, H, W), 'bilinear')
        bad = patch_score < 0.5
        nbad = jnp.maximum(jnp.sum(bad), 1).astype(xhat.dtype)
        mean_bad = jnp.sum(jnp.where(bad, xhat, 0.0)) / nbad
        bm = jnp.where(bad, mean_bad, xhat)[:, 0]
        win = 2 * MIN_DIST + 1
        mp = lax.reduce_window(bm, -jnp.inf, lax.max, (1, win, win), (1, 1, 1), 'SAME')
        peak_scores = jnp.where((bm == mp) & (bm > NMS_THR), bm, -jnp.inf)
        top_s, top_i = lax.top_k(peak_scores.reshape(B, -1), K)
        ys = (top_i // W).astype(jnp.int32)
        xs = (top_i % W).astype(jnp.int32)
        valid = top_s > NMS_THR
        coords = jnp.stack([xs, ys], axis=-1)
        roi_size = 2.0 * PROP_HALF
        offs = (jnp.arange(POOL, dtype=jnp.float32) + 0.5) * (roi_size / POOL)
        px = (xs.astype(jnp.float32) - PROP_HALF)[..., None] + offs
        py = (ys.astype(jnp.float32) - PROP_HALF)[..., None] + offs

        def sample_one(feat, py_i, px_i):
            yy = jnp.broadcast_to(py_i[:, :, None], (K, POOL, POOL))
            xx = jnp.broadcast_to(px_i[:, None, :], (K, POOL, POOL))
            y0 = jnp.floor(yy); x0 = jnp.floor(xx)
            wy = yy - y0; wx = xx - x0
            y0i = jnp.clip(y0.astype(jnp.int32), 0, H - 1)
            y1i = jnp.clip(y0.astype(jnp.int32) + 1, 0, H - 1)
            x0i = jnp.clip(x0.astype(jnp.int32), 0, W - 1)
            x1i = jnp.clip(x0.astype(jnp.int32) + 1, 0, W - 1)
            g = lambda yi, xi: feat[:, yi, xi]
            return (g(y0i, x0i) * (1 - wy) * (1 - wx) + g(y0i, x1i) * (1 - wy) * wx
                    + g(y1i, x0i) * wy * (1 - wx) + g(y1i, x1i) * wy * wx)

        pooled = jax.vmap(sample_one)(feat_up, py, px)
        pooled = pooled.transpose(0, 2, 1, 3, 4).reshape(B * K, C_UP, POOL, POOL)
        dn = ('NCHW', 'OIHW', 'NCHW')
        from jax import lax as _lax
        h = leaky(_lax.conv_general_dilated(pooled, inputs["qw1"], (1, 1), 'SAME', dimension_numbers=dn) + inputs["qb1"][None, :, None, None])
        h = leaky(_lax.conv_general_dilated(h, inputs["qw2"], (1, 1), 'SAME', dimension_numbers=dn) + inputs["qb2"][None, :, None, None])
        h = leaky(_lax.conv_general_dilated(h, inputs["qw3"], (1, 1), 'SAME', dimension_numbers=dn) + inputs["qb3"][None, :, None, None])
        h = jnp.max(h, axis=(2, 3))
        cls_logits = (h @ inputs["qfc_w"].T + inputs["qfc_b"]).reshape(B, K, NCLS)
        return (np.asarray(cls_logits), np.asarray(coords), np.asarray(top_s), np.asarray(valid))


# --------------------------------------------------------------------------
# Stage A device program: NMS + candidate block-reduction
# --------------------------------------------------------------------------
def _build_stage_a():
    nc = bass.Bass()
    m_in = nc.dram_tensor("m_in", [SLAB, W], f32, kind="ExternalInput")
    idx_in = nc.dram_tensor("idx_in", [256, W], f32, kind="ExternalInput")
    bvals = nc.dram_tensor("bvals", [64, 128], f32, kind="ExternalOutput")
    bidx = nc.dram_tensor("bidx", [64, 128], f32, kind="ExternalOutput")
    mx_d = nc.dram_tensor("mx_d", [SLAB, W], f32)
    cand_d = nc.dram_tensor("cand_d", [256, W], f32)

    with TileContext(nc) as tc:
        with tc.tile_pool(name="sb", bufs=2) as pool, \
             tc.tile_pool(name="sb1", bufs=1) as pool1:
            # ---- phase 1: x-direction 7-window max
            for c0, rows in ((0, 128), (128, 128), (256, 6)):
                pad = pool.tile([128, W + 6], f32, tag="pad")
                nc.vector.memset(pad[:rows, :], NEG)
                nc.sync.dma_start(pad[:rows, 3:W + 3], m_in[c0:c0 + rows, :])
                mx = pool.tile([128, W], f32, tag="mx")
                nc.vector.tensor_max(mx[:rows, :], pad[:rows, 0:W], pad[:rows, 1:W + 1])
                for d in range(2, 7):
                    nc.vector.tensor_max(mx[:rows, :], mx[:rows, :], pad[:rows, d:W + d])
                nc.sync.dma_start(mx_d[c0:c0 + rows, :], mx[:rows, :])
            # ---- phase 2: y-direction 7-window max + candidate mask
            for j0, rows in ((0, 120), (120, 120), (240, 16)):
                acc = pool.tile([128, W], f32, tag="yacc")
                nc.sync.dma_start(acc[:rows, :], mx_d[j0:j0 + rows, :])
                for k in range(1, 7):
                    tk = pool.tile([128, W], f32, tag="yld")
                    nc.sync.dma_start(tk[:rows, :], mx_d[j0 + k:j0 + k + rows, :])
                    nc.vector.tensor_max(acc[:rows, :], acc[:rows, :], tk[:rows, :])
                m_t = pool.tile([128, W], f32, tag="mrow")
                nc.sync.dma_start(m_t[:rows, :], m_in[j0 + 3:j0 + 3 + rows, :])
                e01 = pool.tile([128, W], f32, tag="e01")
                nc.vector.tensor_tensor(out=e01[:rows, :], in0=m_t[:rows, :], in1=acc[:rows, :],
                                        op=mybir.AluOpType.is_equal)
                g01 = pool.tile([128, W], f32, tag="g01")
                nc.vector.tensor_scalar(out=g01[:rows, :], in0=m_t[:rows, :], scalar1=float(NMS_THR),
                                        scalar2=None, op0=mybir.AluOpType.is_gt)
                c5 = pool.tile([128, W], f32, tag="c5")
                nc.vector.tensor_tensor(out=c5[:rows, :], in0=e01[:rows, :], in1=g01[:rows, :],
                                        op=mybir.AluOpType.mult)
                nc.vector.tensor_scalar(out=c5[:rows, :], in0=c5[:rows, :], scalar1=0.5, scalar2=None,
                                        op0=mybir.AluOpType.subtract)
                cand = pool.tile([128, W], f32, tag="cand")
                nc.vector.scalar_tensor_tensor(out=cand[:rows, :], in0=c5[:rows, :], scalar=GATE,
                                               in1=m_t[:rows, :], op0=mybir.AluOpType.mult,
                                               op1=mybir.AluOpType.min)
                nc.sync.dma_start(cand_d[j0:j0 + rows, :], cand[:rows, :])
            # ---- phase 3: 4x4 block max + argmax index
            bl = pool1.tile([64, 2048], f32, tag="bl")
            il = pool1.tile([64, 2048], f32, tag="il")
            for rr in range(4):
                src = cand_d.ap().rearrange("(q four) n -> q four n", four=4)[:, rr, :]
                nc.sync.dma_start(bl[:, rr * 512:(rr + 1) * 512].rearrange("p (a c) -> p a c", c=4),
                                  src.rearrange("q (a c) -> q a c", c=4))
                srci = idx_in.ap().rearrange("(q four) n -> q four n", four=4)[:, rr, :]
                nc.sync.dma_start(il[:, rr * 512:(rr + 1) * 512].rearrange("p (a c) -> p a c", c=4),
                                  srci.rearrange("q (a c) -> q a c", c=4))
            bl4 = bl[:].rearrange("p (b a c) -> p a b c", b=4, c=4)
            bmax = pool1.tile([64, 128], f32, tag="bmax")
            nc.vector.tensor_reduce(out=bmax[:], in_=bl4, axis=mybir.AxisListType.XY,
                                    op=mybir.AluOpType.max)
            nc.sync.dma_start(bvals[:], bmax[:])
            eqt = pool1.tile([64, 2048], f32, tag="eqt")
            eq4 = eqt[:].rearrange("p (b a c) -> p a b c", b=4, c=4)
            bb = bmax[:].to_broadcast([64, 128, 4, 4])
            nc.vector.tensor_tensor(out=eq4, in0=bl4, in1=bb, op=mybir.AluOpType.is_equal)
            nc.vector.tensor_scalar(out=eqt[:], in0=eqt[:], scalar1=0.5, scalar2=None,
                                    op0=mybir.AluOpType.subtract)
            isel = pool1.tile([64, 2048], f32, tag="isel")
            nc.vector.scalar_tensor_tensor(out=isel[:], in0=eqt[:], scalar=-GATE,
                                           in1=il[:], op0=mybir.AluOpType.mult,
                                           op1=mybir.AluOpType.max)
            bai = pool1.tile([64, 128], f32, tag="bai")
            nc.vector.tensor_reduce(out=bai[:], in_=isel[:].rearrange("p (b a c) -> p a b c", b=4, c=4),
                                    axis=mybir.AxisListType.XY, op=mybir.AluOpType.min)
            nc.sync.dma_start(bidx[:], bai[:])

    nc.finalize()
    _fix_multiwait(nc)
    return nc


# --------------------------------------------------------------------------
# Stage B device program: roi gather + pooling + conv head
# --------------------------------------------------------------------------
def _pool_matrix():
    """Shared [121, 49] bilinear pooling matrix (canonical fractional weights)."""
    offs = (np.arange(POOL, dtype=np.float32) + np.float32(0.5)) * np.float32(2.0 * PROP_HALF / POOL)
    u0 = np.floor(offs).astype(np.int64)
    fr = (offs - u0.astype(np.float32)).astype(np.float32)
    M = np.zeros((121, 49), dtype=np.float32)
    for i in range(POOL):
        for j in range(POOL):
            s = i * 7 + j
            for du, wu in ((0, np.float32(1.0) - fr[i]), (1, fr[i])):
                for dv, wv in ((0, np.float32(1.0) - fr[j]), (1, fr[j])):
                    t = (u0[i] + du) * 11 + (u0[j] + dv)
                    M[t, s] += wu * wv
    return M


def _build_stage_b(dt_conv=f32r):
    nc = bass.Bass()
    patches = nc.dram_tensor("patches", [64, NROI * 121], f32, kind="ExternalInput")
    mpool_in = nc.dram_tensor("mpool", [121, 49], f32, kind="ExternalInput")
    qw1_in = nc.dram_tensor("qw1p", [64, 9 * 256], f32, kind="ExternalInput")
    qw2_in = nc.dram_tensor("qw2p", [128, 18 * 256], f32, kind="ExternalInput")
    qw3_in = nc.dram_tensor("qw3p", [128, 18 * 256], f32, kind="ExternalInput")
    qfc_in = nc.dram_tensor("qfcp", [128, 8], f32, kind="ExternalInput")
    qb_in = nc.dram_tensor("qbp", [1, 3 * 256], f32, kind="ExternalInput")   # qb1|qb2|qb3
    qfcb_in = nc.dram_tensor("qfcb", [4, 1], f32, kind="ExternalInput")
    lg_out = nc.dram_tensor("lg", [4, NROI], f32, kind="ExternalOutput")

    NB = NROI // 8     # pooling batches
    NG = NROI // 12    # conv groups

    with TileContext(nc) as tc:
        with tc.tile_pool(name="cst", bufs=1) as cpool, \
             tc.tile_pool(name="wrk", bufs=3) as pool, \
             tc.tile_pool(name="act", bufs=2) as apool, \
             tc.tile_pool(name="ps_t", bufs=1, space="PSUM") as ps_t, \
             tc.tile_pool(name="ps_p", bufs=1, space="PSUM") as ps_p, \
             tc.tile_pool(name="ps_r", bufs=1, space="PSUM") as ps_r, \
             tc.tile_pool(name="ps_c", bufs=2, space="PSUM") as ps_c:
            ident = cpool.tile([128, 128], f32)
            make_identity(nc, ident)
            mp_t = cpool.tile([121, 49], f32, tag="mp")
            nc.sync.dma_start(mp_t[:], mpool_in[:])
            qw1_t = cpool.tile([64, 9 * 256], dt_conv, tag="qw1")
            nc.gpsimd.dma_start(qw1_t[:], qw1_in[:])
            qw2_t = cpool.tile([128, 18 * 256], dt_conv, tag="qw2")
            nc.gpsimd.dma_start(qw2_t[:], qw2_in[:])
            qw3_t = cpool.tile([128, 18 * 256], dt_conv, tag="qw3")
            nc.gpsimd.dma_start(qw3_t[:], qw3_in[:])
            qfc_t = cpool.tile([128, 8], dt_conv, tag="qfc")
            nc.gpsimd.dma_start(qfc_t[:], qfc_in[:])
            qb_t = cpool.tile([1, 3 * 256], dt_conv, tag="qb")
            nc.gpsimd.dma_start(qb_t[:], qb_in[:])
            qfcb_t = cpool.tile([4, 1], f32, tag="qfcb")
            nc.sync.dma_start(qfcb_t[:], qfcb_in[:])
            ones_t = cpool.tile([1, 486], dt_conv, tag="ones")
            nc.vector.memset(ones_t[:].bitcast(f32), 1.0)


            # GUARD elements on each side so tap-shifted contiguous reads stay in-tile
            GUARD = 16
            pooled = cpool.tile([64, GUARD + NROI * 81 + GUARD], dt_conv, tag="pooled")
            nc.vector.memset(pooled[:].bitcast(f32), 0.0)
            h0 = cpool.tile([128, NROI], dt_conv, tag="h0")
            h1 = cpool.tile([128, NROI], dt_conv, tag="h1")
            h_t = (h0, h1)

            def emit_pool_batch(bi):
                batch = pool.tile([121, 512], f32, tag="batch")
                pt8 = pool.tile([64, 8 * 121], f32, tag="pt8")
                nc.sync.dma_start(pt8[:], patches[:, bi * 8 * 121:(bi + 1) * 8 * 121])
                for s in range(8):
                    pt_ps = ps_t.tile([121, 64], f32, tag="tp")
                    nc.tensor.transpose(pt_ps[:], pt8[:, s * 121:(s + 1) * 121], ident[:64, :64])
                    nc.vector.tensor_copy(batch[:, s * 64:(s + 1) * 64], pt_ps[:])
                pl_ps = ps_p.tile([49, 512], f32, tag="pp")
                nc.tensor.matmul(pl_ps[:], lhsT=mp_t[:], rhs=batch[:], start=True, stop=True)
                pl_sb = pool.tile([49, 512], f32, tag="plsb")
                nc.vector.tensor_copy(pl_sb[:], pl_ps[:])
                for s in range(8):
                    r = bi * 8 + s
                    rt_ps = ps_r.tile([64, 49], f32, tag="rp")
                    nc.tensor.transpose(rt_ps[:], pl_sb[:, s * 64:(s + 1) * 64], ident[:49, :49])
                    base = GUARD + r * 81 + 10
                    dst = pooled[:, base:base + 63].rearrange("c (r w) -> c r w", r=7, w=9)[:, :, :7]
                    nc.vector.tensor_copy(dst, rt_ps[:].rearrange("c (r w) -> c r w", r=7, w=7))

            # Conv layers compute the FULL padded 9x9 grid per roi (N = 6*81 = 486
            # per matmul): every tap reads a CONTIGUOUS 486-window at a +-10
            # shifted offset (fp32r matmuls reject non-contiguous moving APs).
            # Ring outputs accumulate garbage and are simply never read.
            VIEW49 = ("c (r a b) -> c r a b", dict(r=6, a=9))

            def valid49(tile_ap, elem_base):
                """[P, 6, 7, 7] view of the 49 valid positions of 6 rois' 81-maps."""
                v = tile_ap[:, elem_base:elem_base + 6 * 81].rearrange("c (r a b) -> c r a b", r=6, a=9)
                return v[:, :, 1:8, 1:8]

            def conv_layer(src_of, wt, wcols_fn, bias_col0, dst_fn, kchunks):
                # one conv layer for a 12-roi group processed as 2 halves of 6
                for hh in range(2):
                    for m in range(2):
                        psum = ps_c.tile([128, 486], f32, tag=f"cm{m}")
                        nc.tensor.matmul(psum[:], lhsT=qb_t[0:1, bias_col0 + 128 * m:bias_col0 + 128 * m + 128],
                                         rhs=ones_t[:], start=True, stop=False)
                        n_mm = kchunks * 9
                        cnt = 0
                        for kk in range(kchunks):
                            tile_ap, base = src_of(kk, hh)    # base = element idx of roi0's 81-map
                            for t in range(9):
                                dy, dx = t // 3, t % 3
                                off = base + 9 * (dy - 1) + (dx - 1)
                                rhs = tile_ap[:, off:off + 486]
                                cnt += 1
                                nc.tensor.matmul(psum[:], lhsT=wt[:, wcols_fn(kk, t, m):wcols_fn(kk, t, m) + 128],
                                                 rhs=rhs, start=False, stop=(cnt == n_mm))
                        dst_fn(hh, m, psum)

            def leaky_to(dst49, psum):
                """dst49: [128, 6, 7, 7] SBUF view. psum: [128, 486] full grid."""
                pv = psum[:].rearrange("c (r a b) -> c r a b", r=6, a=9)[:, :, 1:8, 1:8]
                tmp = pool.tile([128, 294], f32, tag="lk", name=f"lk{nc.next_id()}")
                t4 = tmp[:].rearrange("c (r a b) -> c r a b", r=6, a=7)
                nc.scalar.mul(t4, pv, 0.1)
                nc.vector.tensor_tensor(out=dst49, in0=t4, in1=pv, op=mybir.AluOpType.max)

            ACT_GSZ = GUARD + 12 * 81 + GUARD

            def emit_conv_group(g):
                # conv1: input = pooled rois [12g .. 12g+11]
                act1 = [apool.tile([128, ACT_GSZ], dt_conv, name=f"act1_{g}_{m}", tag=f"a1k{m}") for m in range(2)]
                act2 = [apool.tile([128, ACT_GSZ], dt_conv, name=f"act2_{g}_{m}", tag=f"a2k{m}") for m in range(2)]
                for m in range(2):
                    nc.vector.memset(act1[m][:].bitcast(f32), 0.0)
                    nc.vector.memset(act2[m][:].bitcast(f32), 0.0)

                def dst1(hh, m, psum):
                    leaky_to(valid49(act1[m], GUARD + hh * 6 * 81), psum)

                conv_layer(lambda kk, hh: (pooled, GUARD + (g * 12 + hh * 6) * 81),
                           qw1_t, lambda kk, t, m: t * 256 + 128 * m, 0, dst1, 1)

                def dst2(hh, m, psum):
                    leaky_to(valid49(act2[m], GUARD + hh * 6 * 81), psum)

                conv_layer(lambda kk, hh: (act1[kk], GUARD + hh * 6 * 81),
                           qw2_t, lambda kk, t, m: (kk * 9 + t) * 256 + 128 * m, 256, dst2, 2)

                # conv3 + leaky + spatial max -> h
                def dst3(hh, m, psum):
                    a3 = pool.tile([128, 294], dt_conv, tag="a3")
                    leaky_to(a3[:].rearrange("c (r a b) -> c r a b", r=6, a=7), psum)
                    col = g * 12 + hh * 6
                    nc.vector.tensor_reduce(out=h_t[m][:, col:col + 6],
                                            in_=a3[:].rearrange("p (r q) -> p r q", r=6),
                                            axis=mybir.AxisListType.X, op=mybir.AluOpType.max)

                conv_layer(lambda kk, hh: (act2[kk], GUARD + hh * 6 * 81),
                           qw3_t, lambda kk, t, m: (kk * 9 + t) * 256 + 128 * m, 512, dst3, 2)

            bi = 0
            for g in range(NG):
                while bi * 8 < (g + 1) * 12 and bi < NB:
                    emit_pool_batch(bi)
                    bi += 1
                emit_conv_group(g)
            while bi < NB:
                emit_pool_batch(bi)
                bi += 1

            fc_ps = ps_c.tile([4, NROI], f32, tag="cm0")
            nc.tensor.matmul(fc_ps[:], lhsT=qfc_t[:, 0:4], rhs=h_t[0][:], start=True, stop=False)
            nc.tensor.matmul(fc_ps[:], lhsT=qfc_t[:, 4:8], rhs=h_t[1][:], start=False, stop=True)
            lg_sb = pool.tile([4, NROI], f32, tag="lg")
            nc.vector.tensor_tensor(out=lg_sb[:], in0=fc_ps[:],
                                    in1=qfcb_t[:].to_broadcast([4, NROI]),
                                    op=mybir.AluOpType.add)
            nc.sync.dma_start(lg_out[:], lg_sb[:])

    nc.finalize()
    _fix_multiwait(nc)
    return nc


# --------------------------------------------------------------------------
# host-side kernel orchestration
# --------------------------------------------------------------------------
_cache = {}
last_timings = {}
last_in_maps = {}


def _get_stage_a():
    if "a" not in _cache:
        _cache["a"] = _build_stage_a()
    return _cache["a"]


def _get_stage_b():
    if "b" not in _cache:
        _cache["b"] = _build_stage_b(f32r)
    return _cache["b"]


def _pack_conv_weights(inputs):
    qw1 = inputs["qw1"]  # [256, 64, 3, 3]
    qw2 = inputs["qw2"]
    qw3 = inputs["qw3"]
    qw1p = np.zeros((64, 9 * 256), dtype=np.float32)
    for t in range(9):
        dy, dx = t // 3, t % 3
        qw1p[:, t * 256:(t + 1) * 256] = qw1[:, :, dy, dx].T
    def pack23(qw):
        out = np.zeros((128, 18 * 256), dtype=np.float32)
        for kk in range(2):
            for t in range(9):
                dy, dx = t // 3, t % 3
                out[:, (kk * 9 + t) * 256:(kk * 9 + t + 1) * 256] = qw[:, kk * 128:(kk + 1) * 128, dy, dx].T
        return out
    qw2p = pack23(qw2)
    qw3p = pack23(qw3)
    qfcp = np.ascontiguousarray(inputs["qfc_w"].T.reshape(2, 128, 4).transpose(1, 0, 2).reshape(128, 8))
    qbp = np.concatenate([inputs["qb1"], inputs["qb2"], inputs["qb3"]]).reshape(1, -1).astype(np.float32)
    qfcb = inputs["qfc_b"].reshape(4, 1).astype(np.float32)
    return qw1p, qw2p, qw3p, qfcp, qbp, qfcb


def kernel(**inputs):
    inputs = {k: np.asarray(v) for k, v in inputs.items()}
    xhat = inputs["xhat"].astype(np.float32)          # [B,1,H,W]
    feat_up = inputs["feat_up"].astype(np.float32)     # [B,64,H,W]

    # ---------- host float64 patch-score path ----------
    s64 = _host_score64(inputs["feat_down"], inputs["pw1"], inputs["pb1"], inputs["pw2"],
                        inputs["pb2"], inputs["pw3"], inputs["pb3"], inputs["pfc_w"], inputs["pfc_b"])
    bad = s64 < 0.5                                    # [B,H,W] nominal mask
    uncertain = np.abs(s64 - 0.5) < SCORE_BAND
    xh = xhat[:, 0]                                    # [B,H,W]
    m_full = np.where(bad, np.float32(NEG), xh).astype(np.float32)

    nbad = max(int(bad.sum()), 1)
    mean_bad = float(np.where(bad, xh.astype(np.float64), 0.0).sum() / nbad)

    # ---------- stage A ----------
    nc_a = _get_stage_a()
    in_maps_a = []
    for c in range(N_CORES):
        b, half = c // 2, c % 2
        r0 = half * 256
        slab = np.full((SLAB, W), NEG, dtype=np.float32)
        lo, hi = r0 - 3, r0 + 259
        slo, shi = max(lo, 0), min(hi, H)
        slab[slo - lo:shi - lo, :] = m_full[b, slo:shi, :]
        idx = ((np.arange(r0, r0 + 256)[:, None] * W) + np.arange(W)[None, :]).astype(np.float32)
        in_maps_a.append({"m_in": slab, "idx_in": idx})
    import time as _time
    _t0 = _time.time()
    res_a = run_bass_kernel_spmd(nc_a, in_maps_a, core_ids=list(range(N_CORES)))
    last_timings["stage_a_wall"] = _time.time() - _t0
    last_in_maps["a"] = in_maps_a

    # ---------- host: merge candidates, top-k ----------
    need_fallback = False
    sel_idx = np.zeros((B, K), dtype=np.int64)
    sel_val = np.zeros((B, K), dtype=np.float32)
    for b in range(B):
        vals = np.concatenate([res_a.results[2 * b + h]["bvals"].reshape(-1) for h in range(2)])
        idxs = np.concatenate([res_a.results[2 * b + h]["bidx"].reshape(-1) for h in range(2)])
        keep = vals > NMS_THR
        v = vals[keep]
        ix = idxs[keep].astype(np.int64)
        if v.size < K:
            need_fallback = True
            break
        order = np.lexsort((ix, -v))
        v = v[order]
        ix = ix[order]
        sel_val[b] = v[:K]
        sel_idx[b] = ix[:K]
        # guards
        n_unc = int(uncertain[b].sum())
        ci = min(K + 8 * max(n_unc, 1), v.size - 1)
        cutoff_cons = v[ci]
        if not (v[K - 1] > mean_bad + 1e-3):
            need_fallback = True
            break
        if n_unc:
            uy, ux = np.nonzero(uncertain[b])
            if (xh[b, uy, ux] >= cutoff_cons - 1e-6).any():
                need_fallback = True
                break

    if need_fallback:
        return _full_reference_fallback(inputs)

    ys_all = (sel_idx // W).astype(np.int32)
    xs_all = (sel_idx % W).astype(np.int32)

    # ---------- stage B prep ----------
    nc_b = _get_stage_b()
    Mp = _pool_matrix()
    qw1p, qw2p, qw3p, qfcp, qbp, qfcb = _pack_conv_weights(inputs)
    in_maps_b = []
    for c in range(N_CORES):
        b, half = c // 2, c % 2
        ys = ys_all[b, half * 256:(half + 1) * 256]
        xs = xs_all[b, half * 256:(half + 1) * 256]
        patches = np.zeros((64, NROI * 121), dtype=np.float32)
        fb = feat_up[b]
        for r in range(256):
            y, x = int(ys[r]), int(xs[r])
            if 5 <= y <= H - 6 and 5 <= x <= W - 6:
                patch = fb[:, y - 5:y + 6, x - 5:x + 6]
            else:
                rows = np.clip(np.arange(y - 5, y + 6), 0, H - 1)
                cols = np.clip(np.arange(x - 5, x + 6), 0, W - 1)
                patch = fb[:, rows[:, None], cols[None, :]]   # [64,11,11]
            patches[:, r * 121:(r + 1) * 121] = patch.reshape(64, 121)
        in_maps_b.append({"patches": patches,
                          "mpool": Mp, "qw1p": qw1p, "qw2p": qw2p, "qw3p": qw3p,
                          "qfcp": qfcp, "qbp": qbp, "qfcb": qfcb})
    _t0 = _time.time()
    res_b = run_bass_kernel_spmd(nc_b, in_maps_b, core_ids=list(range(N_CORES)))
    last_timings["stage_b_wall"] = _time.time() - _t0
    last_in_maps["b"] = in_maps_b

    # ---------- assemble outputs ----------
    cls_logits = np.zeros((B, K, NCLS), dtype=np.float32)
    for c in range(N_CORES):
        b, half = c // 2, c % 2
        lg = res_b.results[c]["lg"]            # [4, NROI]
        cls_logits[b, half * 256:(half + 1) * 256] = lg[:, :256].T
    coords = np.stack([xs_all, ys_all], axis=-1).astype(np.int32)
    top_s = sel_val.astype(np.float32)
    valid = top_s > NMS_THR
    return cls_logits, coords, top_s, valid
